# revision 1
# baseline (speedup 1.0000x reference)
"""Trainium2 Bass kernel for nn_DIOU3DLoss (mmcv diff_iou_rotated_3d style).

Self-contained: hardcodes shapes/sharding. kernel(pred, target) takes FULL
inputs [262144, 7] f32, shards the box axis across 8 NeuronCores, runs one
SPMD Bass program, and reduces the per-core partial sums to the scalar mean
loss on the host (the unshard step).

Per box pair, work in box1's frame (box1 axis-aligned). Build the mmcv
polygon vertex candidates (corner-in-box tests both ways + edge x
phantom-edge crossings reproducing the mmcv u-sign quirk), place them in 12
fixed slots [C1_j, C2_j, IP_j] x 4 corner clusters, sort each cluster
locally by cross-product sign around the masked centroid, then compute the
shoelace of the valid vertex cycle via a last-valid-vertex
tensor_tensor_scan in a 13-slot box-major layout plus a wrap term from a
reversed scan.
"""

import numpy as np

import concourse.bass as bass
import concourse.tile as tile
from concourse import mybir
from concourse.bass_utils import run_bass_kernel_spmd

P = 128
NCORES = 8
PI = float(np.pi)
TINY = 1e-20
TOL = 1e-6
LOSS_EPS = 1e-6
F32 = mybir.dt.float32
Alu = mybir.AluOpType
Act = mybir.ActivationFunctionType
AxX = mybir.AxisListType.X


def _ap(t, off, dims):
    base = t[:, :]
    return bass.AP(base.tensor, base.offset + off, [base.ap[0]] + dims)


def _legalize_sync(nc):
    """Split multi-wait instructions: this walrus build encodes at most one
    sem-wait (+ one update) per instruction, but Tile's scheduler emits
    several. Carry the extra waits on preceding same-engine NoOps."""
    k = 0
    for fn in nc.m.functions:
        for bl in fn.blocks:
            il = bl.instructions
            new = []
            for inst in il:
                si = getattr(inst, "sync_info", None)
                if si is not None and si.on_wait and len(si.on_wait) > 1:
                    waits = list(si.on_wait)
                    for w in waits[:-1]:
                        k += 1
                        nop = mybir.InstNoOp(name=f"WSPLIT-{k}", ins=[],
                                             outs=[])
                        nop.engine = inst.engine
                        nop.sync_info = mybir.SyncInfo(on_wait=[w],
                                                       on_update=[])
                        new.append(nop)
                    inst.sync_info = mybir.SyncInfo(
                        on_wait=[waits[-1]],
                        on_update=list(si.on_update or []))
                new.append(inst)
            il[:] = new


def build_nc(F, legalize=True):
    """Bass program for one core's shard of P*F boxes.

    DRAM in: pred/target [7, P, F] f32 (param-major, host-transposed).
    DRAM out: out [P, 1] f32, partial sum of (ratio - iou) over the shard.
    """
    nc = bass.Bass(trn_type="TRN2")
    pred_d = nc.dram_tensor("pred", [7, P, F], F32, kind="ExternalInput")
    targ_d = nc.dram_tensor("target", [7, P, F], F32, kind="ExternalInput")
    out_d = nc.dram_tensor("out", [P, 1], F32, kind="ExternalOutput")

    F12 = 12 * F
    F13 = 13 * F + 1

    import contextlib

    with tile.TileContext(nc) as tc, contextlib.ExitStack() as ctx:
        pool = ctx.enter_context(tc.tile_pool(name="main", bufs=1))
        V = nc.vector
        A = nc.scalar
        G = nc.gpsimd

        def tF(tag, w=1):
            return pool.tile([P, w * F], F32, tag=tag, name=tag)

        # ---- load inputs: one DMA per param (keeps DMA-sem fan-in low) ----
        ins = {}
        for name, dram in (("p", pred_d), ("t", targ_d)):
            big = pool.tile([P, 7 * F], F32, tag=f"in_{name}", name=f"in_{name}")
            for i in range(7):
                nc.sync.dma_start(big[:, i * F:(i + 1) * F], dram[i])
                ins[f"{name}{i}"] = big[:, i * F:(i + 1) * F]
        x1, y1, z1 = ins["p0"], ins["p1"], ins["p2"]
        w1, h1, l1, ang1 = ins["p3"], ins["p4"], ins["p5"], ins["p6"]
        x2, y2, z2 = ins["t0"], ins["t1"], ins["t2"]
        w2, h2, l2, ang2 = ins["t3"], ins["t4"], ins["t5"], ins["t6"]

        # ---- trig: range reduce (G) + Sin (A) ----
        trig = {}
        for nm, at in (("1", ang1), ("2", ang2)):
            m = tF(f"trm{nm}")
            ar = tF(f"tra{nm}")
            sh = tF(f"trs{nm}")
            V.tensor_scalar(m[:, :], at[:, :], PI, None, Alu.is_ge)
            V.scalar_tensor_tensor(ar[:, :], m[:, :], -2 * PI, at[:, :],
                                   Alu.mult, Alu.add)
            V.tensor_scalar(m[:, :], ar[:, :], -PI, None, Alu.is_lt)
            V.scalar_tensor_tensor(ar[:, :], m[:, :], 2 * PI, ar[:, :],
                                   Alu.mult, Alu.add)
            V.tensor_scalar(m[:, :], ar[:, :], PI / 2, None, Alu.is_ge)
            V.scalar_tensor_tensor(sh[:, :], m[:, :], -2 * PI, ar[:, :],
                                   Alu.mult, Alu.add)
            V.tensor_scalar(sh[:, :], sh[:, :], PI / 2, None, Alu.add)
            s_ = tF(f"sin{nm}")
            c_ = tF(f"cos{nm}")
            A.activation(s_[:, :], ar[:, :], Act.Sin)
            A.activation(c_[:, :], sh[:, :], Act.Sin)
            trig[f"s{nm}"] = s_
            trig[f"c{nm}"] = c_
        c1t, s1t = trig["c1"], trig["s1"]
        c2t, s2t = trig["c2"], trig["s2"]

        # ---- delta trig + tiny-offset safety ----
        q1, q2 = tF("q1"), tF("q2")
        ct, st = tF("ct"), tF("st")
        V.tensor_mul(q1[:, :], c1t[:, :], c2t[:, :])
        V.tensor_mul(q2[:, :], s1t[:, :], s2t[:, :])
        V.tensor_add(ct[:, :], q1[:, :], q2[:, :])
        V.tensor_mul(q1[:, :], s2t[:, :], c1t[:, :])
        V.tensor_mul(q2[:, :], c2t[:, :], s1t[:, :])
        V.tensor_sub(st[:, :], q1[:, :], q2[:, :])
        for v_ in (ct, st):
            V.tensor_scalar(q1[:, :], v_[:, :], 0.0, None, Alu.is_ge)
            V.scalar_tensor_tensor(v_[:, :], q1[:, :], 2 * TINY, v_[:, :],
                                   Alu.mult, Alu.add)
            V.tensor_scalar(v_[:, :], v_[:, :], TINY, None, Alu.subtract)

        # ---- halfdims ----
        hd = {}
        for nm, src in (("a", w1), ("b", h1), ("hw2", w2), ("hh2", h2),
                        ("hl1", l1), ("hl2", l2)):
            d = tF(f"hd_{nm}")
            A.mul(d[:, :], src[:, :], 0.5)
            hd[nm] = d
        a, b = hd["a"], hd["b"]
        hw2, hh2 = hd["hw2"], hd["hh2"]

        # ---- U, V axis vectors of box2 in frame1 ----
        Ux, Uy, Vx, Vy = tF("Ux"), tF("Uy"), tF("Vx"), tF("Vy")
        V.tensor_mul(Ux[:, :], hw2[:, :], ct[:, :])
        V.tensor_mul(Uy[:, :], hw2[:, :], st[:, :])
        V.scalar_tensor_tensor(Vx[:, :], hh2[:, :], -1.0, st[:, :],
                               Alu.mult, Alu.mult)
        V.tensor_mul(Vy[:, :], hh2[:, :], ct[:, :])

        # ---- o = R(-a1)(c2 - c1) ----
        dxc, dyc = tF("dxc"), tF("dyc")
        ox, oy = tF("ox"), tF("oy")
        A.copy(dxc[:, :], x2[:, :])
        V.tensor_sub(dxc[:, :], dxc[:, :], x1[:, :])
        A.copy(dyc[:, :], y2[:, :])
        V.tensor_sub(dyc[:, :], dyc[:, :], y1[:, :])
        V.tensor_mul(q1[:, :], dxc[:, :], c1t[:, :])
        V.tensor_mul(q2[:, :], dyc[:, :], s1t[:, :])
        V.tensor_add(ox[:, :], q1[:, :], q2[:, :])
        V.tensor_mul(q1[:, :], dxc[:, :], s1t[:, :])
        V.tensor_mul(q2[:, :], dyc[:, :], c1t[:, :])
        V.tensor_sub(oy[:, :], q2[:, :], q1[:, :])

        # ---- master slot tiles (slot-major, 12 slots) ----
        VX = pool.tile([P, F12], F32, tag="VX", name="VX")
        VY = pool.tile([P, F12], F32, tag="VY", name="VY")
        MM = pool.tile([P, F12], F32, tag="MM", name="MM")

        def slots(t, s0, n=4, step=3):
            return _ap(t, s0 * F, [[step * F, n], [1, F]])

        def bc4(t):
            return _ap(t, 0, [[0, 4], [1, F]])

        def sl1(t, s):
            return _ap(t, s * F, [[1, F]])

        def r4(t):
            return _ap(t, 0, [[F, 4], [1, F]])

        QXv, QYv = slots(VX, 1), slots(VY, 1)   # C2 slots 1,4,7,10
        PXv, PYv = slots(VX, 0), slots(VY, 0)   # C1 slots 0,3,6,9

        # box2 corners in frame1 -> C2 slots
        tx1, tx2 = tF("tx1"), tF("tx2")
        sgn = [(1, 1), (-1, 1), (-1, -1), (1, -1)]
        for T_, o_, U_, V_ in ((VX, ox, Ux, Vx), (VY, oy, Uy, Vy)):
            V.tensor_add(tx1[:, :], o_[:, :], U_[:, :])
            V.tensor_sub(tx2[:, :], o_[:, :], U_[:, :])
            for j, (su, sv_) in enumerate(sgn):
                src = tx1 if su > 0 else tx2
                dst = sl1(T_, 3 * j + 1)
                if sv_ > 0:
                    V.tensor_add(dst, src[:, :], V_[:, :])
                else:
                    V.tensor_sub(dst, src[:, :], V_[:, :])

        # box1 corners (+-a, +-b) -> C1 slots
        na, nb = tF("na"), tF("nb")
        A.mul(na[:, :], a[:, :], -1.0)
        A.mul(nb[:, :], b[:, :], -1.0)
        for j, (su, sv_) in enumerate(sgn):
            A.copy(sl1(VX, 3 * j), (a if su > 0 else na)[:, :])
            A.copy(sl1(VY, 3 * j), (b if sv_ > 0 else nb)[:, :])

        # ---- m21: c2 corners inside box1 ----
        t4a, t4b = tF("t4a", 4), tF("t4b", 4)
        ia, ib = tF("ia"), tF("ib")
        infl = 1.0 + 2.0 * TOL
        A.mul(ia[:, :], a[:, :], infl)
        A.mul(ib[:, :], b[:, :], infl)
        A.activation(r4(t4a), QXv, Act.Abs)
        V.tensor_tensor(r4(t4a), r4(t4a), bc4(ia), Alu.is_lt)
        A.activation(r4(t4b), QYv, Act.Abs)
        V.tensor_tensor(r4(t4b), r4(t4b), bc4(ib), Alu.is_lt)
        V.tensor_tensor(slots(MM, 1), r4(t4a), r4(t4b), Alu.mult)

        # ---- m12: c1 corners inside box2 (frame2 coords) ----
        relx, rely = tF("relx", 4), tF("rely", 4)
        xi, eta = tF("xi", 4), tF("eta", 4)
        V.tensor_tensor(r4(relx), PXv, bc4(ox), Alu.subtract)
        V.tensor_tensor(r4(rely), PYv, bc4(oy), Alu.subtract)
        V.tensor_tensor(r4(xi), r4(relx), bc4(ct), Alu.mult)
        V.tensor_tensor(r4(t4a), r4(rely), bc4(st), Alu.mult)
        V.tensor_add(r4(xi), r4(xi), r4(t4a))
        V.tensor_tensor(r4(eta), r4(rely), bc4(ct), Alu.mult)
        V.tensor_tensor(r4(t4a), r4(relx), bc4(st), Alu.mult)
        V.tensor_sub(r4(eta), r4(eta), r4(t4a))
        ia2, ib2 = tF("ia2"), tF("ib2")
        A.mul(ia2[:, :], hw2[:, :], infl)
        A.mul(ib2[:, :], hh2[:, :], infl)
        A.activation(r4(t4a), r4(xi), Act.Abs)
        V.tensor_tensor(r4(t4a), r4(t4a), bc4(ia2), Alu.is_lt)
        A.activation(r4(t4b), r4(eta), Act.Abs)
        V.tensor_tensor(r4(t4b), r4(t4b), bc4(ib2), Alu.is_lt)
        V.tensor_tensor(slots(MM, 0), r4(t4a), r4(t4b), Alu.mult)

        # ---- ipts: box1 edge j x box2 phantom edge k ----
        DX, DY = tF("DX", 4), tF("DY", 4)
        for k, (src_, sg_) in enumerate(((Ux, -2.0), (Vx, -2.0),
                                         (Ux, 2.0), (Vx, 2.0))):
            A.mul(sl1(DX, k), src_[:, :], sg_)
        for k, (src_, sg_) in enumerate(((Uy, -2.0), (Vy, -2.0),
                                         (Uy, 2.0), (Vy, 2.0))):
            A.mul(sl1(DY, k), src_[:, :], sg_)
        rDX, rDY = tF("rDX", 4), tF("rDY", 4)
        V.reciprocal(rDX[:, :2 * F], DX[:, :2 * F])
        A.mul(rDX[:, 2 * F:], rDX[:, :2 * F], -1.0)
        V.reciprocal(rDY[:, :2 * F], DY[:, :2 * F])
        A.mul(rDY[:, 2 * F:], rDY[:, :2 * F], -1.0)

        # axis-paired: pair 0 = edges {0,2} (horiz), pair 1 = edges {1,3}
        sj8 = pool.tile([P, 13 * F + 1], F32, tag="WXM", name="sj8")
        cc8 = pool.tile([P, 13 * F + 1], F32, tag="TMp", name="cc8")
        mk8 = pool.tile([P, 13 * F + 1], F32, tag="xi", name="mk8")
        ab8 = pool.tile([P, 13 * F + 1], F32, tag="TWX", name="ab8")
        ph8 = pool.tile([P, 13 * F + 1], F32, tag="TWY", name="ph8")
        levh = pool.tile([P, 2 * F], F32, tag="levh", name="levh")
        levv = pool.tile([P, 2 * F], F32, tag="levv", name="levv")
        ipm_all = pool.tile([P, 4 * F], F32, tag="eta", name="eta")
        A.copy(levh[:, :F], b[:, :])
        A.copy(levh[:, F:], nb[:, :])
        A.copy(levv[:, :F], na[:, :])
        A.copy(levv[:, F:], a[:, :])

        def r8(t):
            return _ap(t, 0, [[4 * F, 2], [F, 4], [1, F]])

        def bc2x(t, step0, n0):
            return _ap(t, 0, [[step0, 2], [0 if n0 else F, 4], [1, F]])

        for p_ in range(2):
            horiz = p_ == 0
            lev2 = levh if horiz else levv
            Qc = _ap(VY if horiz else VX, F, [[0, 2], [3 * F, 4], [1, F]])
            Qo = _ap(VX if horiz else VY, F, [[0, 2], [3 * F, 4], [1, F]])
            rD = _ap(rDY if horiz else rDX, 0, [[0, 2], [F, 4], [1, F]])
            Do = _ap(DX if horiz else DY, 0, [[0, 2], [F, 4], [1, F]])
            lev_b = _ap(lev2, 0, [[F, 2], [0, 4], [1, F]])
            lim_b = _ap(a if horiz else b, 0, [[0, 2], [0, 4], [1, F]])
            V.tensor_tensor(r8(sj8), lev_b, Qc, Alu.subtract)
            V.tensor_tensor(r8(sj8), r8(sj8), rD, Alu.mult)
            V.tensor_tensor(r8(cc8), r8(sj8), Do, Alu.mult)
            V.tensor_tensor(r8(cc8), r8(cc8), Qo, Alu.add)
            A.activation(r8(ab8), r8(cc8), Act.Abs)
            V.tensor_tensor(r8(ab8), r8(ab8), lim_b, Alu.is_lt)
            V.scalar_tensor_tensor(r8(ph8), r8(sj8), 1.0, r8(sj8),
                                   Alu.add, Alu.mult)
            V.tensor_scalar(r8(ph8), r8(ph8), 0.0, None, Alu.is_lt)
            V.tensor_tensor(r8(mk8), r8(ab8), r8(ph8), Alu.mult)
            for e_ in range(2):
                j = (0 if horiz else 1) + 2 * e_
                base = e_ * 4 * F
                vslot = sl1(VX if horiz else VY, 3 * j + 2)
                oslot = sl1(VY if horiz else VX, 3 * j + 2)
                A.copy(vslot, _ap(cc8, base + 3 * F, [[1, F]]))
                for k in (2, 1, 0):
                    V.copy_predicated(
                        vslot,
                        _ap(mk8, base + k * F, [[1, F]]).bitcast(
                            mybir.dt.int32),
                        _ap(cc8, base + k * F, [[1, F]]))
                A.copy(oslot, lev2[:, e_ * F:(e_ + 1) * F])
                V.tensor_reduce(sl1(ipm_all, j),
                                _ap(mk8, base, [[1, F], [F, 4]]),
                                AxX, Alu.max)
        V.tensor_copy(slots(MM, 2), r4(ipm_all))

        # ---- centroid, center, zero invalid ----
        def r12(t):
            return _ap(t, 0, [[1, F], [F, 12]])

        def s12(t):
            return _ap(t, 0, [[F, 12], [1, F]])

        def bc12(t):
            return _ap(t, 0, [[0, 12], [1, F]])

        WXM = pool.tile([P, F12], F32, tag="WXM", name="WXM")
        WYM = pool.tile([P, F12], F32, tag="xi", name="WYM")
        SX, SY, NV = tF("SX"), tF("SY"), tF("NV")
        G.tensor_mul(WXM[:, :], VX[:, :], MM[:, :])
        G.tensor_mul(WYM[:, :], VY[:, :], MM[:, :])
        V.tensor_reduce(NV[:, :], r12(MM), AxX, Alu.add)
        V.tensor_reduce(SX[:, :], r12(WXM), AxX, Alu.add)
        V.tensor_reduce(SY[:, :], r12(WYM), AxX, Alu.add)
        V.tensor_scalar(NV[:, :], NV[:, :], 1.0, None, Alu.max)
        rNV = tF("rNV")
        V.reciprocal(rNV[:, :], NV[:, :])
        CX, CY = tF("CX"), tF("CY")
        V.tensor_mul(CX[:, :], SX[:, :], rNV[:, :])
        V.tensor_mul(CY[:, :], SY[:, :], rNV[:, :])

        # centered+zeroed vertices written directly into box-major 13-slot
        # layout (col0 = zero pad; box f slot k at col 1+13f+k; k=12 dummy)
        TWX = pool.tile([P, F13], F32, tag="TWX", name="TWX")
        TWY = pool.tile([P, F13], F32, tag="TWY", name="TWY")
        TMp = pool.tile([P, F13], F32, tag="TMp", name="TMp")
        G.memset(TWX[:, :], 0.0)
        G.memset(TWY[:, :], 0.0)
        G.memset(TMp[:, :], 0.0)

        def bm(t, off=1):
            return _ap(t, off, [[13, F], [1, 12]])

        def bcF(t):
            return _ap(t, 0, [[1, F], [0, 12]])

        def r12T(t):
            return _ap(t, 0, [[1, F], [F, 12]])

        V.tensor_tensor(bm(TWX), r12T(VX), bcF(CX), Alu.subtract)
        V.tensor_tensor(bm(TWX), bm(TWX), r12T(MM), Alu.mult)
        V.tensor_tensor(bm(TWY), r12T(VY), bcF(CY), Alu.subtract)
        V.tensor_tensor(bm(TWY), bm(TWY), r12T(MM), Alu.mult)
        V.tensor_scalar(bm(TMp), r12T(MM), -1.0, 1.0, Alu.mult, Alu.add)

        # ---- local 3-sort per cluster (box-major strided views) ----
        cr1 = pool.tile([P, 4 * F], F32, tag="relx", name="relx")
        cr2 = pool.tile([P, 4 * F], F32, tag="rely", name="rely")
        mws = pool.tile([P, 4 * F], F32, tag="xi", name="xi")
        dsw = pool.tile([P, 4 * F], F32, tag="eta", name="eta")

        def bm4(t, s0):
            return _ap(t, 1 + s0, [[13, F], [3, 4]])

        def v4(t):
            return _ap(t, 0, [[4, F], [1, 4]])

        SA = pool.tile([P, F13], F32, tag="VX", name="SA")
        SB = pool.tile([P, F13], F32, tag="VY", name="SB")

        def comp(sa, sb):
            Ax_, Bx_ = bm4(TWX, sa), bm4(TWX, sb)
            Ay_, By_ = bm4(TWY, sa), bm4(TWY, sb)
            crA, crB, msk = bm4(SA, 0), bm4(SA, 1), bm4(SA, 2)
            bkx, bky = bm4(SB, 0), bm4(SB, 1)
            V.tensor_tensor(crA, Ax_, By_, Alu.mult)
            V.tensor_tensor(crB, Ay_, Bx_, Alu.mult)
            V.tensor_tensor(crA, crA, crB, Alu.subtract)
            V.tensor_scalar(msk, crA, 0.0, None, Alu.is_lt)
            mwi = msk.bitcast(mybir.dt.int32)
            A.copy(bkx, Ax_)
            A.copy(bky, Ay_)
            V.copy_predicated(Ax_, mwi, Bx_)
            V.copy_predicated(Bx_, mwi, bkx)
            V.copy_predicated(Ay_, mwi, By_)
            V.copy_predicated(By_, mwi, bky)

        comp(1, 2)

        # ---- scans + shoelace ----
        LX = pool.tile([P, F13], F32, tag="VX", name="VX")   # reuse VX buffer
        LY = pool.tile([P, F13], F32, tag="VY", name="VY")
        V.tensor_tensor_scan(LX[:, :], TMp[:, :], TWX[:, :], 0.0,
                             Alu.mult, Alu.add)
        V.tensor_tensor_scan(LY[:, :], TMp[:, :], TWY[:, :], 0.0,
                             Alu.mult, Alu.add)
        C12 = pool.tile([P, F12], F32, tag="MM", name="MM")  # reuse MM buffer
        SC2 = pool.tile([P, F12], F32, tag="WXM", name="WXM")
        V.tensor_tensor(r12(C12), bm(LX, 0), bm(TWY), Alu.mult)
        V.tensor_tensor(r12(SC2), bm(LY, 0), bm(TWX), Alu.mult)
        V.tensor_sub(C12[:, :], C12[:, :], SC2[:, :])
        AREA2 = pool.tile([P, F], F32, tag="CX", name="CX")
        V.tensor_reduce(AREA2[:, :], r12(C12), AxX, Alu.add)
        # wrap term via reversed scans (negative-step input APs)
        RLX = pool.tile([P, F13], F32, tag="MM", name="MM")
        RLY = pool.tile([P, F13], F32, tag="WXM", name="WXM")

        def rev(t):
            return _ap(t, F13 - 1, [[-1, F13]])

        V.tensor_tensor_scan(RLX[:, :], rev(TMp), rev(TWX), 0.0,
                             Alu.mult, Alu.add)
        V.tensor_tensor_scan(RLY[:, :], rev(TMp), rev(TWY), 0.0,
                             Alu.mult, Alu.add)
        # first valid of box f = RL at rev col F13-2-13f; last = L at 1+13f+11
        V.tensor_tensor(q1[:, :], _ap(LX, 12, [[13, F]]),
                        _ap(RLY, F13 - 2, [[-13, F]]), Alu.mult)
        V.tensor_tensor(q2[:, :], _ap(LY, 12, [[13, F]]),
                        _ap(RLX, F13 - 2, [[-13, F]]), Alu.mult)
        V.tensor_sub(q1[:, :], q1[:, :], q2[:, :])
        V.tensor_add(AREA2[:, :], AREA2[:, :], q1[:, :])
        AREA = pool.tile([P, F], F32, tag="CY", name="CY")
        A.activation(AREA[:, :], AREA2[:, :], Act.Abs, scale=0.5)

        # ---- z overlap / vols / iou ----
        hl1, hl2 = hd["hl1"], hd["hl2"]
        zx1 = pool.tile([P, F], F32, tag="trm1", name="trm1")
        zn1 = pool.tile([P, F], F32, tag="tra1", name="tra1")
        zx2 = pool.tile([P, F], F32, tag="trs1", name="trs1")
        zn2 = pool.tile([P, F], F32, tag="trm2", name="trm2")
        G.tensor_copy(zx1[:, :], z1[:, :])
        G.tensor_add(zx1[:, :], zx1[:, :], hl1[:, :])
        G.tensor_sub(zn1[:, :], z1[:, :], hl1[:, :])
        G.tensor_copy(zx2[:, :], z2[:, :])
        G.tensor_add(zx2[:, :], zx2[:, :], hl2[:, :])
        G.tensor_sub(zn2[:, :], z2[:, :], hl2[:, :])
        V.tensor_tensor(zx1[:, :], zx1[:, :], zx2[:, :], Alu.min)
        V.tensor_max(zn1[:, :], zn1[:, :], zn2[:, :])
        V.tensor_sub(zx1[:, :], zx1[:, :], zn1[:, :])
        V.tensor_scalar(zx1[:, :], zx1[:, :], 0.0, None, Alu.max)  # zo
        vol1 = pool.tile([P, F], F32, tag="tra2", name="tra2")
        vol2 = pool.tile([P, F], F32, tag="trs2", name="trs2")
        G.tensor_copy(vol1[:, :], w1[:, :])
        G.tensor_mul(vol1[:, :], vol1[:, :], h1[:, :])
        G.tensor_mul(vol1[:, :], vol1[:, :], l1[:, :])
        G.tensor_copy(vol2[:, :], w2[:, :])
        G.tensor_mul(vol2[:, :], vol2[:, :], h2[:, :])
        G.tensor_mul(vol2[:, :], vol2[:, :], l2[:, :])
        G.tensor_add(vol1[:, :], vol1[:, :], vol2[:, :])
        inter3 = pool.tile([P, F], F32, tag="in_p0", name="in_p0")
        V.tensor_copy(inter3[:, :], AREA[:, :])
        V.tensor_mul(inter3[:, :], inter3[:, :], zx1[:, :])
        den = pool.tile([P, F], F32, tag="in_p1", name="in_p1")
        V.tensor_sub(den[:, :], vol1[:, :], inter3[:, :])
        rden = pool.tile([P, F], F32, tag="in_t0", name="in_t0")
        V.reciprocal(rden[:, :], den[:, :])
        iou = pool.tile([P, F], F32, tag="in_t1", name="in_t1")
        V.tensor_mul(iou[:, :], inter3[:, :], rden[:, :])

        # ---- ctd + cnd + did + loss ----
        dzc = pool.tile([P, F], F32, tag="in_p6", name="in_p6")
        G.tensor_sub(dzc[:, :], z1[:, :], z2[:, :])
        sq1 = pool.tile([P, F], F32, tag="q1", name="q1")
        sq2 = pool.tile([P, F], F32, tag="q2", name="q2")
        ctd = pool.tile([P, F], F32, tag="dxc", name="dxc")
        A.activation(sq1[:, :], dxc[:, :], Act.Square)
        A.activation(sq2[:, :], dyc[:, :], Act.Square)
        G.tensor_add(ctd[:, :], sq1[:, :], sq2[:, :])
        A.activation(sq1[:, :], dzc[:, :], Act.Square)
        G.tensor_add(ctd[:, :], ctd[:, :], sq1[:, :])
        pr = {}
        for nm, (d_, t_) in (("p11", (a, c1t)), ("p12", (hl1, s1t)),
                             ("p13", (a, s1t)), ("p14", (hl1, c1t)),
                             ("p21", (hw2, c2t)), ("p22", (hl2, s2t)),
                             ("p23", (hw2, s2t)), ("p24", (hl2, c2t))):
            prtag = {"p11": "Ux", "p12": "Uy", "p13": "Vx", "p14": "Vy",
                     "p21": "ox", "p22": "oy", "p23": "dyc", "p24": "SX"}[nm]
            tl = pool.tile([P, F], F32, tag=prtag, name=prtag)
            G.tensor_mul(tl[:, :], d_[:, :], t_[:, :])
            pr[nm] = tl
        quad = pool.tile([P, F], F32, tag="tx2", name="tx2")
        gg = pool.tile([P, F], F32, tag="tx1", name="tx1")
        G.tensor_sub(gg[:, :], pr["p11"][:, :], pr["p21"][:, :])
        A.activation(quad[:, :], gg[:, :], Act.Square)
        for x_, y_ in (("p12", "p22"), ("p23", "p13"), ("p14", "p24")):
            G.tensor_sub(gg[:, :], pr[x_][:, :], pr[y_][:, :])
            A.activation(sq1[:, :], gg[:, :], Act.Square)
            G.tensor_add(quad[:, :], quad[:, :], sq1[:, :])
        G.tensor_sub(gg[:, :], b[:, :], hh2[:, :])
        A.activation(sq1[:, :], gg[:, :], Act.Square)
        G.tensor_add(quad[:, :], quad[:, :], sq1[:, :])
        did = pool.tile([P, F], F32, tag="na", name="na")
        A.activation(sq1[:, :], w2[:, :], Act.Square)
        A.activation(sq2[:, :], h2[:, :], Act.Square)
        G.tensor_add(did[:, :], sq1[:, :], sq2[:, :])
        A.activation(sq1[:, :], l2[:, :], Act.Square)
        G.tensor_add(did[:, :], did[:, :], sq1[:, :])
        S_ = pool.tile([P, F], F32, tag="nb", name="nb")
        V.scalar_tensor_tensor(S_[:, :], ctd[:, :], 2.0, quad[:, :],
                               Alu.mult, Alu.add)
        den2 = pool.tile([P, F], F32, tag="ia", name="ia")
        V.scalar_tensor_tensor(den2[:, :], did[:, :], LOSS_EPS, S_[:, :],
                               Alu.add, Alu.add)
        rden2 = pool.tile([P, F], F32, tag="ib", name="ib")
        V.reciprocal(rden2[:, :], den2[:, :])
        ratio = pool.tile([P, F], F32, tag="ia2", name="ia2")
        V.tensor_mul(ratio[:, :], S_[:, :], rden2[:, :])
        pl = pool.tile([P, F], F32, tag="ib2", name="ib2")
        partial = pool.tile([P, 1], F32, tag="partial", name="partial")
        V.scalar_tensor_tensor(pl[:, :], iou[:, :], -1.0, ratio[:, :],
                               Alu.mult, Alu.add, accum_out=partial[:, :])
        nc.sync.dma_start(out_d[:, :], partial[:, :])

    if legalize:
        _legalize_sync(nc)
    return nc


_NC_CACHE = {}


def _get_nc(F):
    if F not in _NC_CACHE:
        _NC_CACHE[F] = build_nc(F)
    return _NC_CACHE[F]


def kernel(pred: np.ndarray, target: np.ndarray) -> np.ndarray:
    N = pred.shape[0]
    per_core = N // NCORES
    F = per_core // P
    nc = _get_nc(F)
    in_maps = []
    for c in range(NCORES):
        sl = slice(c * per_core, (c + 1) * per_core)
        pm = np.ascontiguousarray(
            pred[sl].astype(np.float32).T.reshape(7, P, F))
        tm = np.ascontiguousarray(
            target[sl].astype(np.float32).T.reshape(7, P, F))
        in_maps.append({"pred": pm, "target": tm})
    res = run_bass_kernel_spmd(nc, in_maps, core_ids=list(range(NCORES)))
    total = 0.0
    for r in res.results:
        total += float(np.sum(r["out"].astype(np.float64)))
    return np.float32(1.0 + total / N)



# revision 3
# speedup vs baseline: 1.1899x; 1.1899x over previous
"""Trainium2 Bass kernel for nn_DIOU3DLoss (mmcv diff_iou_rotated_3d style).

Self-contained: hardcodes shapes/sharding. kernel(pred, target) takes FULL
inputs [262144, 7] f32, shards the box axis across 8 NeuronCores, runs one
SPMD Bass program, and reduces the per-core partial sums to the scalar mean
loss on the host (the unshard step).

Per box pair, work in box1's frame (box1 axis-aligned). Build the mmcv
polygon vertex candidates (corner-in-box tests both ways + edge x
phantom-edge crossings reproducing the mmcv u-sign quirk), place them in 12
fixed slots [C1_j, C2_j, IP_j] x 4 corner clusters, sort each cluster
locally by cross-product sign around the masked centroid, then compute the
shoelace of the valid vertex cycle via a last-valid-vertex
tensor_tensor_scan in a 13-slot box-major layout plus a wrap term from a
reversed scan.

The geometry core runs in float16: packed-f16 tensor_tensor ops get the DVE
2x perf mode and tensor_scalar the 4x mode, roughly halving the vector-
engine time of the polygon pipeline. All coordinates are box1-frame local
(|v| <= ~10), well within f16 range; the masked-slot machinery is arranged
so no Inf*0 NaNs can reach the scan (IP slots are zero-initialized and only
written through the crossing mask).
"""

import numpy as np

import concourse.bass as bass
import concourse.tile as tile
from concourse import mybir
from concourse.bass_utils import run_bass_kernel_spmd

P = 128
NCORES = 8
PI = float(np.pi)
TINY = 1e-20
TINY16 = 2e-3
TOL = 1e-6
LOSS_EPS = 1e-6
F32 = mybir.dt.float32
F16 = mybir.dt.float16
I16 = mybir.dt.int16
Alu = mybir.AluOpType
Act = mybir.ActivationFunctionType
AxX = mybir.AxisListType.X


def _ap(t, off, dims):
    base = t[:, :]
    return bass.AP(base.tensor, base.offset + off, [base.ap[0]] + dims)


def _legalize_sync(nc):
    """Split multi-wait instructions: this walrus build encodes at most one
    sem-wait (+ one update) per instruction, but Tile's scheduler emits
    several. Carry the extra waits on preceding same-engine NoOps."""
    k = 0
    for fn in nc.m.functions:
        for bl in fn.blocks:
            il = bl.instructions
            new = []
            for inst in il:
                si = getattr(inst, "sync_info", None)
                if si is not None and si.on_wait and len(si.on_wait) > 1:
                    waits = list(si.on_wait)
                    for w in waits[:-1]:
                        k += 1
                        nop = mybir.InstNoOp(name=f"WSPLIT-{k}", ins=[],
                                             outs=[])
                        nop.engine = inst.engine
                        nop.sync_info = mybir.SyncInfo(on_wait=[w],
                                                       on_update=[])
                        new.append(nop)
                    inst.sync_info = mybir.SyncInfo(
                        on_wait=[waits[-1]],
                        on_update=list(si.on_update or []))
                new.append(inst)
            il[:] = new


def build_nc(F, legalize=True):
    """Bass program for one core's shard of P*F boxes.

    DRAM in: pred/target [7, P, F] f32 (param-major, host-transposed).
    DRAM out: out [P, 1] f32, partial sum of (ratio - iou) over the shard.
    """
    nc = bass.Bass(trn_type="TRN2")
    pred_d = nc.dram_tensor("pred", [7, P, F], F32, kind="ExternalInput")
    targ_d = nc.dram_tensor("target", [7, P, F], F32, kind="ExternalInput")
    out_d = nc.dram_tensor("out", [P, 1], F32, kind="ExternalOutput")

    F12 = 12 * F
    F13 = 13 * F + 1

    import contextlib

    with tile.TileContext(nc) as tc, contextlib.ExitStack() as ctx:
        pool = ctx.enter_context(tc.tile_pool(name="main", bufs=1))
        V = nc.vector
        A = nc.scalar
        G = nc.gpsimd

        def tF(tag, w=1, dt=F32):
            return pool.tile([P, w * F], dt, tag=tag, name=tag)

        def tH(tag, w=1):
            return pool.tile([P, w * F], F16, tag=tag, name=tag)

        # ---- load inputs: one big DMA per tensor ----
        ins = {}
        for name, dram in (("p", pred_d), ("t", targ_d)):
            big = pool.tile([P, 7 * F], F32, tag=f"in_{name}", name=f"in_{name}")
            d0 = dram[0]
            src = bass.AP(d0.tensor, d0.offset, [[F, P], [P * F, 7], [1, F]])
            dstb = big[:, :]
            dst = bass.AP(dstb.tensor, dstb.offset,
                          [dstb.ap[0], [F, 7], [1, F]])
            nc.sync.dma_start(dst, src)
            for i in range(7):
                ins[f"{name}{i}"] = big[:, i * F:(i + 1) * F]
        x1, y1, z1 = ins["p0"], ins["p1"], ins["p2"]
        w1, h1, l1, ang1 = ins["p3"], ins["p4"], ins["p5"], ins["p6"]
        x2, y2, z2 = ins["t0"], ins["t1"], ins["t2"]
        w2, h2, l2, ang2 = ins["t3"], ins["t4"], ins["t5"], ins["t6"]

        # ---- trig: range reduce (V) + Sin (A), f32 ----
        trig = {}
        for nm, at in (("1", ang1), ("2", ang2)):
            m = tF(f"trm{nm}")
            ar = tF(f"tra{nm}")
            sh = tF(f"trs{nm}")
            V.tensor_scalar(m[:, :], at[:, :], PI, None, Alu.is_ge)
            V.scalar_tensor_tensor(ar[:, :], m[:, :], -2 * PI, at[:, :],
                                   Alu.mult, Alu.add)
            V.tensor_scalar(m[:, :], ar[:, :], -PI, None, Alu.is_lt)
            V.scalar_tensor_tensor(ar[:, :], m[:, :], 2 * PI, ar[:, :],
                                   Alu.mult, Alu.add)
            V.tensor_scalar(m[:, :], ar[:, :], PI / 2, None, Alu.is_ge)
            V.scalar_tensor_tensor(sh[:, :], m[:, :], -2 * PI, ar[:, :],
                                   Alu.mult, Alu.add)
            V.tensor_scalar(sh[:, :], sh[:, :], PI / 2, None, Alu.add)
            s_ = tF(f"sin{nm}")
            c_ = tF(f"cos{nm}")
            A.activation(s_[:, :], ar[:, :], Act.Sin)
            A.activation(c_[:, :], sh[:, :], Act.Sin)
            trig[f"s{nm}"] = s_
            trig[f"c{nm}"] = c_
        c1t, s1t = trig["c1"], trig["s1"]
        c2t, s2t = trig["c2"], trig["s2"]

        # ---- delta trig (f32 math, f16 outputs) + tiny-offset safety ----
        q1, q2 = tF("q1"), tF("q2")
        ct, st = tH("ct"), tH("st")
        qh = tH("qh")
        V.tensor_mul(q1[:, :], c1t[:, :], c2t[:, :])
        V.tensor_mul(q2[:, :], s1t[:, :], s2t[:, :])
        V.tensor_add(ct[:, :], q1[:, :], q2[:, :])
        V.tensor_mul(q1[:, :], s2t[:, :], c1t[:, :])
        V.tensor_mul(q2[:, :], c2t[:, :], s1t[:, :])
        V.tensor_sub(st[:, :], q1[:, :], q2[:, :])
        for v_ in (ct, st):
            V.tensor_scalar(qh[:, :], v_[:, :], 0.0, None, Alu.is_ge)
            V.scalar_tensor_tensor(v_[:, :], qh[:, :], 2 * TINY16, v_[:, :],
                                   Alu.mult, Alu.add)
            V.tensor_scalar(v_[:, :], v_[:, :], TINY16, None, Alu.subtract)

        # ---- halfdims: f32 for z/loss path, f16 for geometry ----
        hl1, hl2 = tF("hl1"), tF("hl2")
        A.mul(hl1[:, :], l1[:, :], 0.5)
        A.mul(hl2[:, :], l2[:, :], 0.5)
        a, b = tH("a"), tH("b")
        hw2, hh2 = tH("hw2"), tH("hh2")
        A.mul(a[:, :], w1[:, :], 0.5)
        A.mul(b[:, :], h1[:, :], 0.5)
        A.mul(hw2[:, :], w2[:, :], 0.5)
        A.mul(hh2[:, :], h2[:, :], 0.5)

        # ---- U, V axis vectors of box2 in frame1 (f16) ----
        Ux, Uy, Vx, Vy = tH("Ux"), tH("Uy"), tH("Vx"), tH("Vy")
        V.tensor_mul(Ux[:, :], hw2[:, :], ct[:, :])
        V.tensor_mul(Uy[:, :], hw2[:, :], st[:, :])
        V.scalar_tensor_tensor(Vx[:, :], hh2[:, :], -1.0, st[:, :],
                               Alu.mult, Alu.mult)
        V.tensor_mul(Vy[:, :], hh2[:, :], ct[:, :])

        # ---- o = R(-a1)(c2 - c1): f32 math (dxc/dyc reused by ctd), f16 out
        dxc, dyc = tF("dxc"), tF("dyc")
        ox, oy = tH("ox"), tH("oy")
        V.tensor_sub(dxc[:, :], x2[:, :], x1[:, :])
        V.tensor_sub(dyc[:, :], y2[:, :], y1[:, :])
        V.tensor_mul(q1[:, :], dxc[:, :], c1t[:, :])
        V.tensor_mul(q2[:, :], dyc[:, :], s1t[:, :])
        V.tensor_add(ox[:, :], q1[:, :], q2[:, :])
        V.tensor_mul(q1[:, :], dxc[:, :], s1t[:, :])
        V.tensor_mul(q2[:, :], dyc[:, :], c1t[:, :])
        V.tensor_sub(oy[:, :], q2[:, :], q1[:, :])

        # ---- master slot tiles (slot-major, 12 slots, f16) ----
        VX = pool.tile([P, F12], F16, tag="VX", name="VX")
        VY = pool.tile([P, F12], F16, tag="VY", name="VY")
        MM = pool.tile([P, F12], F16, tag="MM", name="MM")

        def slots(t, s0, n=4, step=3):
            return _ap(t, s0 * F, [[step * F, n], [1, F]])

        def bc4(t):
            return _ap(t, 0, [[0, 4], [1, F]])

        def sl1(t, s):
            return _ap(t, s * F, [[1, F]])

        def r4(t):
            return _ap(t, 0, [[F, 4], [1, F]])

        QXv, QYv = slots(VX, 1), slots(VY, 1)   # C2 slots 1,4,7,10
        PXv, PYv = slots(VX, 0), slots(VY, 0)   # C1 slots 0,3,6,9

        # box2 corners in frame1 -> C2 slots (f16)
        tx1, tx2 = tH("tx1"), tH("tx2")
        sgn = [(1, 1), (-1, 1), (-1, -1), (1, -1)]
        for T_, o_, U_, V_ in ((VX, ox, Ux, Vx), (VY, oy, Uy, Vy)):
            V.tensor_add(tx1[:, :], o_[:, :], U_[:, :])
            V.tensor_sub(tx2[:, :], o_[:, :], U_[:, :])
            for j, (su, sv_) in enumerate(sgn):
                src = tx1 if su > 0 else tx2
                dst = sl1(T_, 3 * j + 1)
                if sv_ > 0:
                    V.tensor_add(dst, src[:, :], V_[:, :])
                else:
                    V.tensor_sub(dst, src[:, :], V_[:, :])

        # box1 corners (+-a, +-b) -> C1 slots
        na, nb = tH("na"), tH("nb")
        V.tensor_scalar(na[:, :], a[:, :], -1.0, None, Alu.mult)
        V.tensor_scalar(nb[:, :], b[:, :], -1.0, None, Alu.mult)
        for j, (su, sv_) in enumerate(sgn):
            A.copy(sl1(VX, 3 * j), (a if su > 0 else na)[:, :])
            A.copy(sl1(VY, 3 * j), (b if sv_ > 0 else nb)[:, :])

        # ---- m21: c2 corners inside box1 (f16) ----
        t4a, t4b = tH("t4a", 4), tH("t4b", 4)
        ia, ib = tH("ia"), tH("ib")
        infl = 1.0 + 2.0 * TOL
        A.mul(ia[:, :], a[:, :], infl)
        A.mul(ib[:, :], b[:, :], infl)
        A.activation(r4(t4a), QXv, Act.Abs)
        V.tensor_tensor(r4(t4a), r4(t4a), bc4(ia), Alu.is_lt)
        A.activation(r4(t4b), QYv, Act.Abs)
        V.tensor_tensor(r4(t4b), r4(t4b), bc4(ib), Alu.is_lt)
        V.tensor_tensor(slots(MM, 1), r4(t4a), r4(t4b), Alu.mult)

        # ---- m12: c1 corners inside box2 (frame2 coords, f16) ----
        relx, rely = tH("relx", 4), tH("rely", 4)
        xi, eta = tH("xi", 4), tH("eta", 4)
        V.tensor_tensor(r4(relx), PXv, bc4(ox), Alu.subtract)
        V.tensor_tensor(r4(rely), PYv, bc4(oy), Alu.subtract)
        V.tensor_tensor(r4(xi), r4(relx), bc4(ct), Alu.mult)
        V.tensor_tensor(r4(t4a), r4(rely), bc4(st), Alu.mult)
        V.tensor_add(r4(xi), r4(xi), r4(t4a))
        V.tensor_tensor(r4(eta), r4(rely), bc4(ct), Alu.mult)
        V.tensor_tensor(r4(t4a), r4(relx), bc4(st), Alu.mult)
        V.tensor_sub(r4(eta), r4(eta), r4(t4a))
        ia2, ib2 = tH("ia2"), tH("ib2")
        A.mul(ia2[:, :], hw2[:, :], infl)
        A.mul(ib2[:, :], hh2[:, :], infl)
        A.activation(r4(t4a), r4(xi), Act.Abs)
        V.tensor_tensor(r4(t4a), r4(t4a), bc4(ia2), Alu.is_lt)
        A.activation(r4(t4b), r4(eta), Act.Abs)
        V.tensor_tensor(r4(t4b), r4(t4b), bc4(ib2), Alu.is_lt)
        V.tensor_tensor(slots(MM, 0), r4(t4a), r4(t4b), Alu.mult)

        # ---- ipts: box1 edge j x box2 phantom edge k ----
        # directions in f32 on Act (reciprocal needs f32), then f16 copies
        DX32, DY32 = tF("DX32", 4), tF("DY32", 4)
        for k, (src_, sg_) in enumerate(((Ux, -2.0), (Vx, -2.0),
                                         (Ux, 2.0), (Vx, 2.0))):
            A.mul(sl1(DX32, k), src_[:, :], sg_)
        for k, (src_, sg_) in enumerate(((Uy, -2.0), (Vy, -2.0),
                                         (Uy, 2.0), (Vy, 2.0))):
            A.mul(sl1(DY32, k), src_[:, :], sg_)
        rDX32, rDY32 = tF("rDX32", 4), tF("rDY32", 4)
        V.reciprocal(rDX32[:, :2 * F], DX32[:, :2 * F])
        A.mul(rDX32[:, 2 * F:], rDX32[:, :2 * F], -1.0)
        V.reciprocal(rDY32[:, :2 * F], DY32[:, :2 * F])
        A.mul(rDY32[:, 2 * F:], rDY32[:, :2 * F], -1.0)
        DX, DY = tH("DX", 4), tH("DY", 4)
        rDX, rDY = tH("rDX", 4), tH("rDY", 4)
        V.tensor_copy(DX[:, :], DX32[:, :])
        V.tensor_copy(DY[:, :], DY32[:, :])
        V.tensor_copy(rDX[:, :], rDX32[:, :])
        V.tensor_copy(rDY[:, :], rDY32[:, :])

        # axis-paired: pair 0 = edges {0,2} (horiz), pair 1 = edges {1,3}
        sj8 = pool.tile([P, F13], F16, tag="WXM", name="sj8")
        cc8 = pool.tile([P, F13], F16, tag="TMp", name="cc8")
        mk8 = pool.tile([P, F13], F16, tag="xi16", name="mk8")
        ab8 = pool.tile([P, F13], F16, tag="TWX", name="ab8")
        ph8 = pool.tile([P, F13], F16, tag="TWY", name="ph8")
        levh = pool.tile([P, 2 * F], F16, tag="levh", name="levh")
        levv = pool.tile([P, 2 * F], F16, tag="levv", name="levv")
        ipm_all = pool.tile([P, 4 * F], F16, tag="eta16", name="ipm")
        A.copy(levh[:, :F], b[:, :])
        A.copy(levh[:, F:], nb[:, :])
        A.copy(levv[:, :F], na[:, :])
        A.copy(levv[:, F:], a[:, :])

        # zero-init the IP slots (3j+2) so all-masked-out lanes stay finite
        G.memset(slots(VX, 2), 0.0)
        G.memset(slots(VY, 2), 0.0)

        def r8(t):
            return _ap(t, 0, [[4 * F, 2], [F, 4], [1, F]])

        for p_ in range(2):
            horiz = p_ == 0
            lev2 = levh if horiz else levv
            Qc = _ap(VY if horiz else VX, F, [[0, 2], [3 * F, 4], [1, F]])
            Qo = _ap(VX if horiz else VY, F, [[0, 2], [3 * F, 4], [1, F]])
            rD = _ap(rDY if horiz else rDX, 0, [[0, 2], [F, 4], [1, F]])
            Do = _ap(DX if horiz else DY, 0, [[0, 2], [F, 4], [1, F]])
            lev_b = _ap(lev2, 0, [[F, 2], [0, 4], [1, F]])
            lim_b = _ap(a if horiz else b, 0, [[0, 2], [0, 4], [1, F]])
            V.tensor_tensor(r8(sj8), lev_b, Qc, Alu.subtract)
            V.tensor_tensor(r8(sj8), r8(sj8), rD, Alu.mult)
            V.tensor_tensor(r8(cc8), r8(sj8), Do, Alu.mult)
            V.tensor_tensor(r8(cc8), r8(cc8), Qo, Alu.add)
            A.activation(r8(ab8), r8(cc8), Act.Abs)
            V.tensor_tensor(r8(ab8), r8(ab8), lim_b, Alu.is_lt)
            # phantom-side test: -1 < sj8 < 0 (two f16 tensor_scalar cmps)
            V.tensor_scalar(r8(ph8), r8(sj8), -1.0, None, Alu.is_gt)
            V.tensor_scalar(r8(mk8), r8(sj8), 0.0, None, Alu.is_lt)
            V.tensor_tensor(r8(ph8), r8(ph8), r8(mk8), Alu.mult)
            V.tensor_tensor(r8(mk8), r8(ab8), r8(ph8), Alu.mult)
            for e_ in range(2):
                j = (0 if horiz else 1) + 2 * e_
                base = e_ * 4 * F
                vslot = sl1(VX if horiz else VY, 3 * j + 2)
                oslot = sl1(VY if horiz else VX, 3 * j + 2)
                for k in (3, 2, 1, 0):
                    V.copy_predicated(
                        vslot,
                        _ap(mk8, base + k * F, [[1, F]]).bitcast(I16),
                        _ap(cc8, base + k * F, [[1, F]]))
                A.copy(oslot, lev2[:, e_ * F:(e_ + 1) * F])
                V.tensor_reduce(sl1(ipm_all, j),
                                _ap(mk8, base, [[1, F], [F, 4]]),
                                AxX, Alu.max)
        V.tensor_copy(slots(MM, 2), r4(ipm_all))

        # ---- centroid, center, zero invalid ----
        def r12(t):
            return _ap(t, 0, [[1, F], [F, 12]])

        def s12(t):
            return _ap(t, 0, [[F, 12], [1, F]])

        def bc12(t):
            return _ap(t, 0, [[0, 12], [1, F]])

        WXM = pool.tile([P, F12], F16, tag="WXM", name="WXM")
        WYM = pool.tile([P, F12], F16, tag="xi16", name="WYM")
        SX, SY, NV = tF("SX"), tF("SY"), tF("NV")
        G.tensor_mul(WXM[:, :], VX[:, :], MM[:, :])
        G.tensor_mul(WYM[:, :], VY[:, :], MM[:, :])
        V.tensor_reduce(NV[:, :], r12(MM), AxX, Alu.add)
        V.tensor_reduce(SX[:, :], r12(WXM), AxX, Alu.add)
        V.tensor_reduce(SY[:, :], r12(WYM), AxX, Alu.add)
        V.tensor_scalar(NV[:, :], NV[:, :], 1.0, None, Alu.max)
        rNV = tF("rNV")
        V.reciprocal(rNV[:, :], NV[:, :])
        CX, CY = tH("CX"), tH("CY")
        V.tensor_mul(CX[:, :], SX[:, :], rNV[:, :])
        V.tensor_mul(CY[:, :], SY[:, :], rNV[:, :])

        # centered+zeroed vertices written directly into box-major 13-slot
        # layout (col0 = zero pad; box f slot k at col 1+13f+k; k=12 dummy)
        TWX = pool.tile([P, F13], F16, tag="TWX", name="TWX")
        TWY = pool.tile([P, F13], F16, tag="TWY", name="TWY")
        TMp = pool.tile([P, F13], F16, tag="TMp", name="TMp")
        G.memset(TWX[:, :], 0.0)
        G.memset(TWY[:, :], 0.0)
        G.memset(TMp[:, :], 0.0)

        def bm(t, off=1):
            return _ap(t, off, [[13, F], [1, 12]])

        def bcF(t):
            return _ap(t, 0, [[1, F], [0, 12]])

        def r12T(t):
            return _ap(t, 0, [[1, F], [F, 12]])

        V.tensor_tensor(bm(TWX), r12T(VX), bcF(CX), Alu.subtract)
        V.tensor_tensor(bm(TWX), bm(TWX), r12T(MM), Alu.mult)
        V.tensor_tensor(bm(TWY), r12T(VY), bcF(CY), Alu.subtract)
        V.tensor_tensor(bm(TWY), bm(TWY), r12T(MM), Alu.mult)
        V.tensor_scalar(bm(TMp), r12T(MM), -1.0, 1.0, Alu.mult, Alu.add)

        # ---- local 3-sort per cluster (box-major strided views) ----
        def bm4(t, s0):
            return _ap(t, 1 + s0, [[13, F], [3, 4]])

        SA = pool.tile([P, F13], F16, tag="VX", name="SA")
        SB = pool.tile([P, F13], F16, tag="VY", name="SB")

        def comp(sa, sb):
            Ax_, Bx_ = bm4(TWX, sa), bm4(TWX, sb)
            Ay_, By_ = bm4(TWY, sa), bm4(TWY, sb)
            crA, crB, msk = bm4(SA, 0), bm4(SA, 1), bm4(SA, 2)
            bkx, bky = bm4(SB, 0), bm4(SB, 1)
            V.tensor_tensor(crA, Ax_, By_, Alu.mult)
            V.tensor_tensor(crB, Ay_, Bx_, Alu.mult)
            V.tensor_tensor(crA, crA, crB, Alu.subtract)
            V.tensor_scalar(msk, crA, 0.0, None, Alu.is_lt)
            mwi = msk.bitcast(I16)
            A.copy(bkx, Ax_)
            A.copy(bky, Ay_)
            V.copy_predicated(Ax_, mwi, Bx_)
            V.copy_predicated(Bx_, mwi, bkx)
            V.copy_predicated(Ay_, mwi, By_)
            V.copy_predicated(By_, mwi, bky)

        comp(1, 2)

        # ---- scans + shoelace ----
        LX = pool.tile([P, F13], F16, tag="VX", name="LX")
        LY = pool.tile([P, F13], F16, tag="VY", name="LY")
        V.tensor_tensor_scan(LX[:, :], TMp[:, :], TWX[:, :], 0.0,
                             Alu.mult, Alu.add)
        V.tensor_tensor_scan(LY[:, :], TMp[:, :], TWY[:, :], 0.0,
                             Alu.mult, Alu.add)
        C12 = pool.tile([P, F13], F16, tag="MM", name="C12")
        SC2 = pool.tile([P, F13], F16, tag="WXM", name="SC2")
        V.tensor_tensor(bm(C12), bm(LX, 0), bm(TWY), Alu.mult)
        V.tensor_tensor(bm(SC2), bm(LY, 0), bm(TWX), Alu.mult)
        V.tensor_tensor(bm(C12), bm(C12), bm(SC2), Alu.subtract)
        AREA2 = pool.tile([P, F], F32, tag="CX32", name="AREA2")
        V.tensor_reduce(AREA2[:, :], bm(C12), AxX, Alu.add)
        # wrap term via reversed scans (negative-step input APs)
        RLX = pool.tile([P, F13], F16, tag="MM", name="RLX")
        RLY = pool.tile([P, F13], F16, tag="WXM", name="RLY")

        def rev(t):
            return _ap(t, F13 - 1, [[-1, F13]])

        V.tensor_tensor_scan(RLX[:, :], rev(TMp), rev(TWX), 0.0,
                             Alu.mult, Alu.add)
        V.tensor_tensor_scan(RLY[:, :], rev(TMp), rev(TWY), 0.0,
                             Alu.mult, Alu.add)
        # first valid of box f = RL at rev col F13-2-13f; last = L at 1+13f+11
        V.tensor_tensor(q1[:, :], _ap(LX, 12, [[13, F]]),
                        _ap(RLY, F13 - 2, [[-13, F]]), Alu.mult)
        V.tensor_tensor(q2[:, :], _ap(LY, 12, [[13, F]]),
                        _ap(RLX, F13 - 2, [[-13, F]]), Alu.mult)
        V.tensor_sub(q1[:, :], q1[:, :], q2[:, :])
        V.tensor_add(AREA2[:, :], AREA2[:, :], q1[:, :])
        AREA = pool.tile([P, F], F32, tag="CY32", name="AREA")
        A.activation(AREA[:, :], AREA2[:, :], Act.Abs, scale=0.5)

        # ---- z overlap / vols / iou (f32) ----
        zx1 = pool.tile([P, F], F32, tag="trm1", name="zx1")
        zn1 = pool.tile([P, F], F32, tag="tra1", name="zn1")
        zx2 = pool.tile([P, F], F32, tag="trs1", name="zx2")
        zn2 = pool.tile([P, F], F32, tag="trm2", name="zn2")
        G.tensor_copy(zx1[:, :], z1[:, :])
        G.tensor_add(zx1[:, :], zx1[:, :], hl1[:, :])
        G.tensor_sub(zn1[:, :], z1[:, :], hl1[:, :])
        G.tensor_copy(zx2[:, :], z2[:, :])
        G.tensor_add(zx2[:, :], zx2[:, :], hl2[:, :])
        G.tensor_sub(zn2[:, :], z2[:, :], hl2[:, :])
        V.tensor_tensor(zx1[:, :], zx1[:, :], zx2[:, :], Alu.min)
        V.tensor_max(zn1[:, :], zn1[:, :], zn2[:, :])
        V.tensor_sub(zx1[:, :], zx1[:, :], zn1[:, :])
        V.tensor_scalar(zx1[:, :], zx1[:, :], 0.0, None, Alu.max)  # zo
        vol1 = pool.tile([P, F], F32, tag="tra2", name="vol1")
        vol2 = pool.tile([P, F], F32, tag="trs2", name="vol2")
        G.tensor_copy(vol1[:, :], w1[:, :])
        G.tensor_mul(vol1[:, :], vol1[:, :], h1[:, :])
        G.tensor_mul(vol1[:, :], vol1[:, :], l1[:, :])
        G.tensor_copy(vol2[:, :], w2[:, :])
        G.tensor_mul(vol2[:, :], vol2[:, :], h2[:, :])
        G.tensor_mul(vol2[:, :], vol2[:, :], l2[:, :])
        G.tensor_add(vol1[:, :], vol1[:, :], vol2[:, :])
        inter3 = pool.tile([P, F], F32, tag="in_p0", name="inter3")
        V.tensor_copy(inter3[:, :], AREA[:, :])
        V.tensor_mul(inter3[:, :], inter3[:, :], zx1[:, :])
        den = pool.tile([P, F], F32, tag="in_p1", name="den")
        V.tensor_sub(den[:, :], vol1[:, :], inter3[:, :])
        rden = pool.tile([P, F], F32, tag="in_t0", name="rden")
        V.reciprocal(rden[:, :], den[:, :])
        iou = pool.tile([P, F], F32, tag="in_t1", name="iou")
        V.tensor_mul(iou[:, :], inter3[:, :], rden[:, :])

        # ---- ctd + cnd + did + loss (f32) ----
        dzc = pool.tile([P, F], F32, tag="in_p6", name="dzc")
        G.tensor_sub(dzc[:, :], z1[:, :], z2[:, :])
        sq1 = pool.tile([P, F], F32, tag="q1", name="sq1")
        sq2 = pool.tile([P, F], F32, tag="q2", name="sq2")
        ctd = pool.tile([P, F], F32, tag="dxc", name="ctd")
        A.activation(sq1[:, :], dxc[:, :], Act.Square)
        A.activation(sq2[:, :], dyc[:, :], Act.Square)
        G.tensor_add(ctd[:, :], sq1[:, :], sq2[:, :])
        A.activation(sq1[:, :], dzc[:, :], Act.Square)
        G.tensor_add(ctd[:, :], ctd[:, :], sq1[:, :])
        # quad terms: (0.5*w1*c1 - 0.5*w2*c2)^2 etc, f32 on Pool/Act
        a32 = pool.tile([P, F], F32, tag="a32", name="a32")
        hw232 = pool.tile([P, F], F32, tag="hw232", name="hw232")
        A.mul(a32[:, :], w1[:, :], 0.5)
        A.mul(hw232[:, :], w2[:, :], 0.5)
        pr = {}
        for nm, (d_, t_) in (("p11", (a32, c1t)), ("p12", (hl1, s1t)),
                             ("p13", (a32, s1t)), ("p14", (hl1, c1t)),
                             ("p21", (hw232, c2t)), ("p22", (hl2, s2t)),
                             ("p23", (hw232, s2t)), ("p24", (hl2, c2t))):
            prtag = {"p11": "pt1", "p12": "pt2", "p13": "pt3", "p14": "pt4",
                     "p21": "pt5", "p22": "pt6", "p23": "pt7",
                     "p24": "pt8"}[nm]
            tl = pool.tile([P, F], F32, tag=prtag, name=prtag)
            G.tensor_mul(tl[:, :], d_[:, :], t_[:, :])
            pr[nm] = tl
        quad = pool.tile([P, F], F32, tag="tx2f", name="quad")
        gg = pool.tile([P, F], F32, tag="tx1f", name="gg")
        G.tensor_sub(gg[:, :], pr["p11"][:, :], pr["p21"][:, :])
        A.activation(quad[:, :], gg[:, :], Act.Square)
        for x_, y_ in (("p12", "p22"), ("p23", "p13"), ("p14", "p24")):
            G.tensor_sub(gg[:, :], pr[x_][:, :], pr[y_][:, :])
            A.activation(sq1[:, :], gg[:, :], Act.Square)
            G.tensor_add(quad[:, :], quad[:, :], sq1[:, :])
        G.tensor_sub(gg[:, :], h1[:, :], h2[:, :])
        A.activation(sq1[:, :], gg[:, :], Act.Square, scale=0.5)
        G.tensor_add(quad[:, :], quad[:, :], sq1[:, :])
        did = pool.tile([P, F], F32, tag="na32", name="did")
        A.activation(sq1[:, :], w2[:, :], Act.Square)
        A.activation(sq2[:, :], h2[:, :], Act.Square)
        G.tensor_add(did[:, :], sq1[:, :], sq2[:, :])
        A.activation(sq1[:, :], l2[:, :], Act.Square)
        G.tensor_add(did[:, :], did[:, :], sq1[:, :])
        S_ = pool.tile([P, F], F32, tag="nb32", name="S_")
        V.scalar_tensor_tensor(S_[:, :], ctd[:, :], 2.0, quad[:, :],
                               Alu.mult, Alu.add)
        den2 = pool.tile([P, F], F32, tag="ia32", name="den2")
        V.scalar_tensor_tensor(den2[:, :], did[:, :], LOSS_EPS, S_[:, :],
                               Alu.add, Alu.add)
        rden2 = pool.tile([P, F], F32, tag="ib32", name="rden2")
        V.reciprocal(rden2[:, :], den2[:, :])
        ratio = pool.tile([P, F], F32, tag="ia2f", name="ratio")
        V.tensor_mul(ratio[:, :], S_[:, :], rden2[:, :])
        pl = pool.tile([P, F], F32, tag="ib2f", name="pl")
        partial = pool.tile([P, 1], F32, tag="partial", name="partial")
        V.scalar_tensor_tensor(pl[:, :], iou[:, :], -1.0, ratio[:, :],
                               Alu.mult, Alu.add, accum_out=partial[:, :])
        nc.sync.dma_start(out_d[:, :], partial[:, :])

    if legalize:
        _legalize_sync(nc)
    return nc


_NC_CACHE = {}


def _get_nc(F):
    if F not in _NC_CACHE:
        _NC_CACHE[F] = build_nc(F)
    return _NC_CACHE[F]


def kernel(pred: np.ndarray, target: np.ndarray) -> np.ndarray:
    N = pred.shape[0]
    per_core = N // NCORES
    F = per_core // P
    nc = _get_nc(F)
    in_maps = []
    for c in range(NCORES):
        sl = slice(c * per_core, (c + 1) * per_core)
        pm = np.ascontiguousarray(
            pred[sl].astype(np.float32).T.reshape(7, P, F))
        tm = np.ascontiguousarray(
            target[sl].astype(np.float32).T.reshape(7, P, F))
        in_maps.append({"pred": pm, "target": tm})
    res = run_bass_kernel_spmd(nc, in_maps, core_ids=list(range(NCORES)))
    total = 0.0
    for r in res.results:
        total += float(np.sum(r["out"].astype(np.float64)))
    return np.float32(1.0 + total / N)
